# revision 7
# baseline (speedup 1.0000x reference)
"""Block-convolution kernel for trn2 (8 NeuronCores, SPMD data-parallel over batch).

Problem: seq_vector [16, 4096, 512] f32, W [7, 512, 512, 7], b [7, 512].
Each block of 8 sequence positions: out position 1+i = conv of kernel size
i+1 (taps 0..i of the block) with weights W[i]; position 0 is zero.

Formulation: one GEMM per output block-slot i:
  Y_i[m, o] = sum_{tap<=i, e} XT[(tap, e), m] * G_i[(tap, e), o] + b[i, o]
with m = (batch, block) flattened. Data-parallel: 2 of 16 batch rows per core.

Device layout (per core):
  XT   [28, 128, 1024]  - X transposed, k-tile major (k = tap*512 + e)
  G    [112, 128, 512]  - masked weights, per-i blocks of (i+1)*4 k-tiles
  BREP [128, 3584]      - bias replicated across partitions
  OUT  [1024, 8, 512]   - per (block-row, position, channel); pos 0 untouched
"""

import numpy as np
from contextlib import ExitStack

N, S, E = 16, 4096, 512
K = 7           # taps / conv count
BS = 8          # block size
B = S // BS     # 512 blocks per sequence
NCORES = 8
NPC = N // NCORES          # batches per core = 2
M = NPC * B                # 1024 rows per core
KT_TOT = K * (E // 128)    # 28 contraction k-tiles of 128
MT = M // 128              # 8 m-tiles

# Compute dtype for the matmuls: "float32" (exact, 1/4 PE rate) or
# "float32r" / "float16" / "bfloat16" (full PE rate, reduced precision).
MODE = "float32"

_CACHE = {}


def _goff(i):
    # row offset (in 128-row k-tiles) of output-block i inside G
    return 4 * (i * (i + 1) // 2)


def _build_nc(mode):
    import concourse.mybir as mybir
    import concourse.tile as tile
    from concourse import bacc

    mdt = getattr(mybir.dt, mode)
    f32 = mybir.dt.float32

    nc = bacc.Bacc("TRN2", target_bir_lowering=False, debug=False)
    xt_d = nc.dram_tensor("xt", [KT_TOT, 128, M], mdt, kind="ExternalInput")
    g_d = nc.dram_tensor("g", [4 * _goff(K - 1) // 4 + 4 * K, 128, E], mdt,
                         kind="ExternalInput")  # [112, 128, 512]
    br_d = nc.dram_tensor("brep", [128, K * E], f32, kind="ExternalInput")
    out_d = nc.dram_tensor("out", [M, BS, E], f32, kind="ExternalOutput")

    with tile.TileContext(nc) as tc, ExitStack() as ctx:
        xt_pool = ctx.enter_context(tc.tile_pool(name="xt", bufs=K))
        g_pool = ctx.enter_context(tc.tile_pool(name="g", bufs=4))
        bias_pool = ctx.enter_context(tc.tile_pool(name="bias", bufs=1))
        out_pool = ctx.enter_context(tc.tile_pool(name="out", bufs=6))
        psum_pool = ctx.enter_context(tc.tile_pool(name="ps", bufs=8, space="PSUM"))

        bias_sb = bias_pool.tile([128, K * E], f32)
        nc.sync.dma_start(bias_sb[:], br_d.ap()[:, :])

        # Zero out position 0 of every block (the reference leaves it zero).
        zt = bias_pool.tile([128, E], f32)
        nc.vector.memset(zt[:], 0.0)
        for m in range(MT):
            nc.sync.dma_start(out_d.ap()[m * 128 : (m + 1) * 128, 0, :], zt[:])

        # X resident: one tile per tap, [128, 4 k-subtiles x 1024 m].
        # The DMA for tap t is emitted at its first use (i == t) so the
        # program-order DMA issue matches the consumption order.
        xt_sb = [None] * K

        for i in range(K):
            nkt = 4 * (i + 1)
            psums = [psum_pool.tile([128, E], f32, name="ps", tag="ps") for _ in range(MT)]
            for tap in range(i + 1):
                if xt_sb[tap] is None:
                    t = xt_pool.tile([128, 4 * M], mdt, name="xtt", tag="xtt")
                    nc.sync.dma_start(
                        t[:].rearrange("p (s m) -> p s m", s=4),
                        xt_d.ap()[4 * tap : 4 * tap + 4, :, :].rearrange("s p m -> p s m"),
                    )
                    xt_sb[tap] = t
                g_sb = g_pool.tile([128, 4 * E], mdt)
                kt0 = _goff(i) + 4 * tap
                nc.sync.dma_start(
                    g_sb[:].rearrange("p (s o) -> p s o", s=4),
                    g_d.ap()[kt0 : kt0 + 4, :, :].rearrange("s p o -> p s o"),
                )
                for sub in range(4):
                    kt = 4 * tap + sub
                    for m in range(MT):
                        nc.tensor.matmul(
                            psums[m][:],
                            xt_sb[tap][:, sub * M + m * 128 : sub * M + (m + 1) * 128],
                            g_sb[:, sub * E : (sub + 1) * E],
                            start=(kt == 0),
                            stop=(kt == nkt - 1),
                        )
            for m in range(MT):
                ot = out_pool.tile([128, E], f32)
                nc.vector.tensor_add(ot[:], psums[m][:], bias_sb[:, i * E : (i + 1) * E])
                nc.sync.dma_start(out_d.ap()[m * 128 : (m + 1) * 128, i + 1, :], ot[:])

    nc.compile()
    return nc


def _prep_inputs(seq_vector, W, b, mode):
    np_dt = {"float32": np.float32, "float32r": np.float32,
             "float16": np.float16}.get(mode)
    if np_dt is None:
        import ml_dtypes
        np_dt = ml_dtypes.bfloat16

    xb = seq_vector.reshape(N, B, BS, E)[:, :, :K, :]          # [16,512,8->7,512]
    xt_all = np.ascontiguousarray(
        xb.transpose(2, 3, 0, 1).reshape(K * E, N * B), dtype=np_dt
    )                                                          # [3584, 8192]
    g = np.concatenate(
        [W[i].transpose(2, 1, 0)[: i + 1].reshape((i + 1) * E, E) for i in range(K)],
        axis=0,
    ).astype(np_dt)                                            # [14336, 512]
    g = np.ascontiguousarray(g).reshape(112, 128, E)
    brep = np.ascontiguousarray(
        np.broadcast_to(b.reshape(1, K * E), (128, K * E)), dtype=np.float32
    )
    per_core = []
    for c in range(NCORES):
        xt_c = np.ascontiguousarray(
            xt_all[:, c * M : (c + 1) * M]
        ).reshape(KT_TOT, 128, M)
        per_core.append({"xt": xt_c, "g": g, "brep": brep})
    return per_core


def _get_runner(mode):
    """Build (once) and return a callable in_maps -> list of per-core out arrays."""
    key = ("runner", mode)
    if key in _CACHE:
        return _CACHE[key]

    import jax
    from jax.sharding import Mesh, PartitionSpec
    from jax.experimental.shard_map import shard_map
    from concourse import bass2jax
    from concourse.bass2jax import _bass_exec_p
    import concourse.mybir as mybir

    nc = _build_nc(mode)
    bass2jax.install_neuronx_cc_hook()

    partition_name = nc.partition_id_tensor.name if nc.partition_id_tensor else None
    in_names, out_names, out_avals, zero_shapes = [], [], [], []
    for alloc in nc.m.functions[0].allocations:
        if not isinstance(alloc, mybir.MemoryLocationSet):
            continue
        name = alloc.memorylocations[0].name
        if alloc.kind == "ExternalInput":
            if name != partition_name:
                in_names.append(name)
        elif alloc.kind == "ExternalOutput":
            out_names.append(name)
            shape = tuple(alloc.tensor_shape)
            dtype = mybir.dt.np(alloc.dtype)
            out_avals.append(jax.core.ShapedArray(shape, dtype))
            zero_shapes.append((shape, dtype))
    n_params = len(in_names)
    n_outs = len(out_avals)
    all_names = list(in_names) + out_names
    if partition_name is not None:
        all_names.append(partition_name)

    def _body(*args):
        operands = list(args)
        if partition_name is not None:
            operands.append(bass2jax.partition_id_tensor())
        outs = _bass_exec_p.bind(
            *operands,
            out_avals=tuple(out_avals),
            in_names=tuple(all_names),
            out_names=tuple(out_names),
            lowering_input_output_aliases=(),
            sim_require_finite=True,
            sim_require_nnan=True,
            nc=nc,
        )
        return tuple(outs)

    devices = jax.devices()[:NCORES]
    mesh = Mesh(np.asarray(devices), ("core",))
    donate = tuple(range(n_params, n_params + n_outs))
    sharded = jax.jit(
        shard_map(
            _body,
            mesh=mesh,
            in_specs=(PartitionSpec("core"),) * (n_params + n_outs),
            out_specs=(PartitionSpec("core"),) * n_outs,
            check_rep=False,
        ),
        donate_argnums=donate,
        keep_unused=True,
    )

    # The kernel writes every element of the output, so the donated
    # "initial output" buffers are pure placeholders. Build them on-device
    # to avoid shipping zero bytes through the tunnel on every call.
    sharding = jax.sharding.NamedSharding(mesh, PartitionSpec("core"))

    import jax.numpy as jnp

    _zeros_jit = jax.jit(
        lambda: tuple(
            jnp.zeros((NCORES * s[0], *s[1:]), d) for (s, d) in zero_shapes
        ),
        out_shardings=tuple(sharding for _ in zero_shapes),
    )

    def _dev_zeros():
        return list(_zeros_jit())

    def run(per_core_inputs, timing_iters=0):
        concat_in = [
            np.concatenate([m[name] for m in per_core_inputs], axis=0)
            for name in in_names
        ]
        in_dev = [jax.device_put(a, sharding) for a in concat_in]
        out_arrs = sharded(*in_dev, *_dev_zeros())
        if timing_iters:
            import time

            for a in out_arrs:
                a.block_until_ready()
            times = []
            for _ in range(timing_iters):
                t0 = time.perf_counter()
                out_arrs = sharded(*in_dev, *out_arrs)
                for a in out_arrs:
                    a.block_until_ready()
                times.append(time.perf_counter() - t0)
            run.last_times = times
        out = np.asarray(out_arrs[0])
        return out.reshape(NCORES, *out_avals[0].shape)

    _CACHE[key] = run
    return run


def kernel(seq_vector, W, b):
    seq_vector = np.asarray(seq_vector, dtype=np.float32)
    W = np.asarray(W, dtype=np.float32)
    b = np.asarray(b, dtype=np.float32)
    run = _get_runner(MODE)
    per_core = _prep_inputs(seq_vector, W, b, MODE)
    outs = run(per_core)                       # [8, 1024, 8, 512]
    return np.ascontiguousarray(outs.reshape(N, S, E))
